# revision 8
# baseline (speedup 1.0000x reference)
"""Trainium2 Bass kernel for nn_Capsule_2731599200537 (capsule dynamic routing).

Reference computation:
    votes[b,i,j,a] = sum_k x[b,i,k] * W[i,k,j*A+a]
    3 rounds of leaky-softmax routing over j; output = squash(preact_3).

Strategy (8 NeuronCores, shard i = N_IN across cores; W/x sharded by i, routing
batch-local, 128KB AllReduce of preact partials per round):
  - votes kept on-chip in fp16, layout [(j,a)-partition chunks, (b, i) free].
  - votes produced by dense (i,k)=128-row contraction per 16-i tile:
    lhsT = W chunk [128, 128], rhs = block-diag x [128, (b,i')] built with a
    mask multiply (out = votes for 16 i x 128 (ja) x 32 b per matmul).
  - round-1 preact is route-uniform: single dense (ik)-contraction matmul.
  - dlogit (votes . act) on PE: votes as stationary, block-diag act as moving.
  - preact (route . votes) bypasses stored votes: xr = x*route on DVE, then
    per-(j,k) matmuls against W [i-part, (k,ja)] accumulate in PSUM.
"""
import numpy as np

import concourse.bacc as bacc
import concourse.mybir as mybir
from concourse import tile
from concourse.bass_utils import run_bass_kernel_spmd
from concourse.masks import make_identity

B = 64          # batch
NI = 2048       # input capsules
KA = 8          # input atoms
NO = 32         # output capsules
AT = 16         # output atoms
O = NO * AT     # 512
R = 3           # routing iterations
NCORES = 8
NIS = NI // NCORES   # 256 input capsules per core
NH = NIS // 128      # 2 i-halves of 128
NT = NIS // 16       # 16 (i,k)-chunks of 128 rows (16 i's each)

F32 = mybir.dt.float32
F16 = mybir.dt.float16
AF = mybir.ActivationFunctionType
AX = mybir.AxisListType


def build(n_cores: int = NCORES, use_collective: bool = True,
          cc_rounds=(1, 2, 3), serialize_cc: bool = True):
    nc = bacc.Bacc(None, target_bir_lowering=False, debug=False,
                   num_devices=n_cores)
    x_d = nc.dram_tensor("x", [B, NIS, KA], F32, kind="ExternalInput")
    w_d = nc.dram_tensor("w", [NIS, KA, O], F32, kind="ExternalInput")
    b_d = nc.dram_tensor("b", [NO, AT], F32, kind="ExternalInput")
    y_d = nc.dram_tensor("y", [B, NO, AT], F32, kind="ExternalOutput")

    wik = w_d.rearrange("i k o -> (i k) o")    # [2048, 512]
    wko = w_d.rearrange("i k o -> i (k o)")    # [256, 4096]

    evict_n = [0]
    last_inst = {}

    def evict(dst, src):
        """PSUM->SBUF copies, alternated across DVE/ACT to split the load."""
        if evict_n[0] % 2 == 0:
            last_inst["v"] = nc.vector.tensor_copy(dst, src)
        else:
            last_inst["s"] = nc.scalar.copy(dst, src)
        evict_n[0] += 1

    with tile.TileContext(nc) as tc:
        with (
            tc.tile_pool(name="big", bufs=1) as big,
            tc.tile_pool(name="cst", bufs=1) as cst,
            tc.tile_pool(name="psA", bufs=3, space="PSUM") as psA,
            tc.tile_pool(name="psT", bufs=2, space="PSUM") as psT,
            tc.tile_pool(name="psB", bufs=1, space="PSUM") as psB,
            tc.tile_pool(name="dram", bufs=2, space="DRAM") as dram,
        ):
            # ---- persistent SBUF ----
            votes = [big.tile([128, B * NIS], F16, tag=f"votes{c}", name=f"votes{c}")
                     for c in range(4)]                       # [(ja)c, (b, i)]
            vv = [t.rearrange("p (b i) -> p b i", b=B) for t in votes]
            w2 = [big.tile([128, KA * O], F16, tag=f"w2_{h}", name=f"w2_{h}")
                  for h in range(NH)]                         # [i, (k, ja)]
            w2v = [t.rearrange("p (k o) -> p k o", k=KA) for t in w2]
            xt2k = [big.tile([128, KA * B], F16, tag=f"xt2k{h}", name=f"xt2k{h}")
                    for h in range(NH)]                       # [i, (k, b)]
            xt2kv = [t.rearrange("p (k b) -> p k b", k=KA) for t in xt2k]

            ident = cst.tile([64, 64], F16)
            make_identity(nc, ident[:])
            bias_bc = cst.tile([64, O], F32)
            brow = cst.tile([1, O], F32)
            nc.sync.dma_start(brow[:], b_d.rearrange("j a -> (j a)").unsqueeze(0))
            nc.gpsimd.partition_broadcast(bias_bc[:], brow[:])

            # =========== phase 1: loads, converts, votes production ===========
            with (
                tc.tile_pool(name="p1", bufs=1) as p1,
                tc.tile_pool(name="stg", bufs=3) as stg,
                tc.tile_pool(name="xbp", bufs=4) as xbp,
            ):
                # x -> fp16 [b, (i k)]
                xnat16 = p1.tile([64, NIS * KA], F16)
                for c in range(4):
                    st = stg.tile([128, 512], F32, tag="stage")
                    nc.sync.dma_start(
                        st[:64, :],
                        x_d.rearrange("b i k -> b (i k)")[:, 512 * c:512 * (c + 1)])
                    nc.vector.tensor_copy(
                        xnat16[:, 512 * c:512 * (c + 1)], st[:64, :])

                # W -> fp16 [(i k), ja]  (16 chunks of 128 rows)
                w16 = p1.tile([128, NT * O], F16)
                w16v = w16.rearrange("p (t o) -> p t o", t=NT)
                for t in range(NT):
                    st = stg.tile([128, 512], F32, tag="stage")
                    nc.sync.dma_start(st[:], wik[128 * t:128 * (t + 1), :])
                    evict(w16v[:, t, :], st[:])

                # W -> fp16 [i, (k ja)]  (2 halves of 128 rows)
                for h in range(NH):
                    for c in range(8):
                        st = stg.tile([128, 512], F32, tag="stage")
                        nc.sync.dma_start(
                            st[:], wko[128 * h:128 * (h + 1),
                                       512 * c:512 * (c + 1)])
                        evict(w2[h][:, 512 * c:512 * (c + 1)], st[:])

                # xT16 [(i k), b]: PE transposes of xnat16
                xt16 = p1.tile([128, NT * B], F16)
                xt16v = xt16.rearrange("p (t b) -> p t b", t=NT)
                for t in range(NT):
                    pt = psT.tile([128, 64], F16, tag="ptT")
                    nc.tensor.transpose(pt[:], xnat16[:, 128 * t:128 * (t + 1)],
                                        ident[:])
                    evict(xt16v[:, t, :], pt[:])

                # xt2k [i, (k, b)]: strided-column PE transposes
                xnk = xnat16.rearrange("b (i k) -> b i k", k=KA)
                for h in range(NH):
                    for k in range(KA):
                        pt = psT.tile([128, 64], F16, tag="ptT")
                        nc.tensor.transpose(
                            pt[:], xnk[:, 128 * h:128 * (h + 1), k], ident[:])
                        evict(xt2kv[h][:, k, :], pt[:])

                # block-diagonal selection mask [128, 16]: mask[(i',k), i''] = (i'==i'')
                mask = p1.tile([128, 16], F16)
                nc.gpsimd.memset(mask[:], 1.0)
                nc.gpsimd.affine_select(
                    out=mask[:], in_=mask[:],
                    compare_op=mybir.AluOpType.is_ge, fill=0.0,
                    base=0, pattern=[[-8, 16]], channel_multiplier=1)
                nc.gpsimd.affine_select(
                    out=mask[:], in_=mask[:],
                    compare_op=mybir.AluOpType.is_ge, fill=0.0,
                    base=7, pattern=[[8, 16]], channel_multiplier=-1)

                # ---- votes production ----
                for t in range(NT):
                    xb = xbp.tile([128, B * 16], F16, tag="xb")
                    xbv = xb.rearrange("p (b i) -> p b i", b=B)
                    nc.vector.tensor_mul(
                        xbv,
                        mask.unsqueeze(1).broadcast_to([128, B, 16]),
                        xt16v[:, t, :].unsqueeze(2).broadcast_to([128, B, 16]))
                    for c2 in range(4):
                        for h in range(2):
                            bank = psA.tile([128, 512], F32, tag="bank")
                            last_inst["mm"] = nc.tensor.matmul(
                                bank[:],
                                w16v[:, t, 128 * c2:128 * (c2 + 1)],
                                xb[:, 512 * h:512 * (h + 1)],
                                start=True, stop=True)
                            evict(vv[c2][:, 32 * h:32 * (h + 1),
                                         16 * t:16 * (t + 1)],
                                  bank.rearrange("p (b i) -> p b i", b=32))

                # ---- round-1 preact: uniform route = 1/33 ----
                psb1 = psB.tile([64, 512], F32, tag="psb")
                for t in range(NT):
                    nc.tensor.matmul(psb1[:], xt16v[:, t, :], w16v[:, t, :],
                                     start=(t == 0), stop=(t == NT - 1))

            # =========== routing ===========
            with (
                tc.tile_pool(name="rt", bufs=1) as rt,
                tc.tile_pool(name="xrp", bufs=4) as xrp,
            ):
                logits = [rt.tile([128, B * NO], F32, tag=f"lg{h}", name=f"lg{h}")
                          for h in range(NH)]                 # [i, (b, j)]
                lgv = [t.rearrange("p (b j) -> p b j", b=B) for t in logits]
                routef = [rt.tile([128, NO * B], F16, tag=f"rf{h}", name=f"rf{h}")
                          for h in range(NH)]                 # [i, (j, b)]
                rfv = [t.rearrange("p (j b) -> p j b", j=NO) for t in routef]
                actblk = [rt.tile([128, B * 8], F16, tag=f"ab{c}", name=f"ab{c}")
                          for c in range(4)]                  # [(ja)c, (b, j8)]
                av = [t.rearrange("p (b j) -> p b j", b=B) for t in actblk]
                actT = [rt.tile([128, 64], F16, tag=f"actT{c}", name=f"actT{c}")
                        for c in range(4)]
                # maskJ[(j',a), j8] = (j' == j8)
                maskJ = rt.tile([128, 8], F16, tag="maskJ")
                nc.gpsimd.memset(maskJ[:], 1.0)
                nc.gpsimd.affine_select(
                    out=maskJ[:], in_=maskJ[:],
                    compare_op=mybir.AluOpType.is_ge, fill=0.0,
                    base=0, pattern=[[-16, 8]], channel_multiplier=1)
                nc.gpsimd.affine_select(
                    out=maskJ[:], in_=maskJ[:],
                    compare_op=mybir.AluOpType.is_ge, fill=0.0,
                    base=15, pattern=[[16, 8]], channel_multiplier=-1)

                pre_part = rt.tile([64, O], F32, tag="pre_part")
                pre_sum = rt.tile([64, O], F32, tag="pre_sum")
                sq = rt.tile([64, O], F32, tag="sq")
                nsq = rt.tile([64, NO], F32, tag="nsq")
                norm = rt.tile([64, NO], F32, tag="norm")
                d1 = rt.tile([64, NO], F32, tag="d1")
                rd = rt.tile([64, NO], F32, tag="rd")
                fs = rt.tile([64, NO], F32, tag="fs")
                acts = rt.tile([64, O], F32, tag="acts")
                act16 = rt.tile([64, O], F16, tag="act16")
                denom = rt.tile([128, B], F32, tag="denom")
                recip = rt.tile([128, B], F32, tag="recip")
                recip16 = rt.tile([128, B], F16, tag="recip16")

                for r in range(1, R + 1):
                    if r == 1:
                        # preact partial = (sum_i votes) / 33
                        nc.scalar.mul(pre_part[:], psb1[:], 1.0 / 33.0)
                    else:
                        # ---- actblk from act16 (previous round) ----
                        for c in range(4):
                            pt = psT.tile([128, 64], F16, tag="ptT")
                            nc.tensor.transpose(
                                pt[:], act16[:, 128 * c:128 * (c + 1)], ident[:])
                            evict(actT[c][:], pt[:])
                            nc.vector.tensor_mul(
                                av[c],
                                maskJ.unsqueeze(1).broadcast_to([128, B, 8]),
                                actT[c].unsqueeze(2).broadcast_to([128, B, 8]))

                        # ---- dlogit: logits += votes . act ----
                        for h in range(NH):
                            for bg in range(4):
                                bank = psA.tile([128, 512], F32, tag="bank")
                                for b16 in range(16):
                                    b = 16 * bg + b16
                                    for c in range(4):
                                        off = 32 * b16 + 8 * c
                                        nc.tensor.matmul(
                                            bank[:, off:off + 8],
                                            vv[c][:, b, 128 * h:128 * (h + 1)],
                                            av[c][:, b, :],
                                            start=True, stop=True)
                                lsl = lgv[h][:, 16 * bg:16 * (bg + 1), :]
                                bkv = bank.rearrange("p (b j) -> p b j", b=16)
                                if r == 2:
                                    evict(lsl, bkv)
                                else:
                                    nc.vector.tensor_add(lsl, bkv, lsl)

                        # ---- leaky softmax over j ----
                        for h in range(NH):
                            nc.scalar.activation(
                                rfv[h].transpose([0, 2, 1]), lgv[h], AF.Exp)
                            nc.vector.reduce_sum(
                                denom[:], rfv[h].transpose([0, 2, 1]), axis=AX.X)
                            nc.vector.tensor_scalar_add(denom[:], denom[:], 1.0)
                            nc.vector.reciprocal(recip[:], denom[:])
                            nc.vector.tensor_copy(recip16[:], recip[:])
                            nc.vector.tensor_mul(
                                rfv[h], rfv[h],
                                recip16.unsqueeze(1).broadcast_to([128, NO, B]))

                        # ---- preact: route . votes via xr = x*route vs W ----
                        psb = psB.tile([64, 512], F32, tag="psb")
                        for j in range(NO):
                            for h in range(NH):
                                xr = xrp.tile([128, KA * B], F16, tag="xr")
                                xrv = xr.rearrange("p (k b) -> p k b", k=KA)
                                nc.vector.tensor_mul(
                                    xrv, xt2kv[h],
                                    rfv[h][:, j, :].unsqueeze(1)
                                    .broadcast_to([128, KA, B]))
                                for k in range(KA):
                                    nc.tensor.matmul(
                                        psb[:, 16 * j:16 * (j + 1)],
                                        xr[:, 64 * k:64 * (k + 1)],
                                        w2v[h][:, k, 16 * j:16 * (j + 1)],
                                        start=(h == 0 and k == 0),
                                        stop=(h == NH - 1 and k == KA - 1))
                        nc.scalar.copy(pre_part[:], psb[:])

                    # ---- cross-core reduce of preact partials ----
                    if use_collective and r in cc_rounds:
                        cc_in = dram.tile([64, O], F32, tag="ccin")
                        cc_out = dram.tile([64, O], F32, tag="ccout")
                        nc.sync.dma_start(cc_in[:], pre_part[:])
                        cc = nc.gpsimd.collective_compute(
                            "AllReduce", mybir.AluOpType.add,
                            replica_groups=[list(range(n_cores))],
                            ins=[cc_in.opt()], outs=[cc_out.opt()])
                        if serialize_cc and r == 1:
                            from concourse.tile_rust import add_dep_helper
                            for li in last_inst.values():
                                add_dep_helper(cc.ins, li.ins, True,
                                               "cc after production")
                        nc.sync.dma_start(pre_sum[:], cc_out[:])
                    else:
                        nc.vector.tensor_copy(pre_sum[:], pre_part[:])
                    nc.vector.tensor_add(pre_sum[:], pre_sum[:], bias_bc[:])

                    # ---- squash ----
                    nc.scalar.activation(sq[:], pre_sum[:], AF.Square)
                    nc.vector.reduce_sum(
                        nsq[:], sq.rearrange("p (j a) -> p j a", j=NO), axis=AX.X)
                    nc.scalar.activation(norm[:], nsq[:], AF.Sqrt)
                    nc.vector.tensor_scalar_add(d1[:], nsq[:], 1.0)
                    nc.vector.reciprocal(rd[:], d1[:])
                    nc.vector.tensor_mul(fs[:], norm[:], rd[:])
                    nc.vector.tensor_mul(
                        acts.rearrange("p (j a) -> p j a", j=NO),
                        pre_sum.rearrange("p (j a) -> p j a", j=NO),
                        fs.unsqueeze(2).broadcast_to([64, NO, AT]))

                    if r < R:
                        nc.vector.tensor_copy(act16[:], acts[:])
                    else:
                        nc.sync.dma_start(
                            y_d.rearrange("b j a -> b (j a)"), acts[:])

    nc.compile()
    return nc


_NC_CACHE = {}


def _get_nc(n_cores=NCORES, use_collective=True):
    key = (n_cores, use_collective)
    if key not in _NC_CACHE:
        _NC_CACHE[key] = build(n_cores, use_collective)
    return _NC_CACHE[key]


class Runner:
    """Compiles the Bass module to a PJRT executable once; reusable calls."""

    def __init__(self, nc, n_cores=NCORES):
        import jax
        import concourse.mybir as _mybir
        from concourse import bass2jax as b2j
        from jax.experimental.shard_map import shard_map
        from jax.sharding import Mesh, PartitionSpec

        b2j.install_neuronx_cc_hook()
        self.nc = nc
        self.n_cores = n_cores
        pname = nc.partition_id_tensor.name if nc.partition_id_tensor else None
        in_names, out_names, out_avals, zero_outs = [], [], [], []
        for alloc in nc.m.functions[0].allocations:
            if not isinstance(alloc, _mybir.MemoryLocationSet):
                continue
            name = alloc.memorylocations[0].name
            if alloc.kind == "ExternalInput":
                if name != pname:
                    in_names.append(name)
            elif alloc.kind == "ExternalOutput":
                shape = tuple(alloc.tensor_shape)
                dtype = _mybir.dt.np(alloc.dtype)
                out_names.append(name)
                out_avals.append(jax.core.ShapedArray(shape, dtype))
                zero_outs.append(np.zeros(shape, dtype))
        self.in_names = list(in_names)
        self.out_names = out_names
        self.out_avals = out_avals
        self.zero_outs = zero_outs
        n_params = len(in_names)
        all_names = in_names + out_names + ([pname] if pname else [])
        donate = tuple(range(n_params, n_params + len(out_names)))
        self.n_params = n_params

        def _body(*args):
            operands = list(args)
            if pname is not None:
                operands.append(b2j.partition_id_tensor())
            outs = b2j._bass_exec_p.bind(
                *operands,
                out_avals=tuple(out_avals),
                in_names=tuple(all_names),
                out_names=tuple(out_names),
                lowering_input_output_aliases=(),
                sim_require_finite=False,
                sim_require_nnan=False,
                nc=nc,
            )
            return tuple(outs)

        devices = jax.devices()[:n_cores]
        mesh = Mesh(np.asarray(devices), ("core",))
        nio = n_params + len(out_names)
        self._jit = jax.jit(
            shard_map(_body, mesh=mesh,
                      in_specs=(PartitionSpec("core"),) * nio,
                      out_specs=(PartitionSpec("core"),) * len(out_names),
                      check_rep=False),
            donate_argnums=donate, keep_unused=True)

    def __call__(self, in_maps, block=True):
        n = self.n_cores
        concat_in = [
            np.concatenate([np.asarray(in_maps[c][name]) for c in range(n)],
                           axis=0)
            for name in self.in_names
        ]
        concat_zero = [
            np.zeros((n * z.shape[0], *z.shape[1:]), z.dtype)
            for z in self.zero_outs
        ]
        out = self._jit(*concat_in, *concat_zero)
        if block:
            for o in out:
                o.block_until_ready()
        return [
            {name: np.asarray(out[i]).reshape(n, *self.out_avals[i].shape)[c]
             for i, name in enumerate(self.out_names)}
            for c in range(n)
        ]


_RUNNER_CACHE = {}


def get_runner(n_cores=NCORES, use_collective=True):
    key = (n_cores, use_collective)
    if key not in _RUNNER_CACHE:
        _RUNNER_CACHE[key] = Runner(_get_nc(n_cores, use_collective), n_cores)
    return _RUNNER_CACHE[key]


def make_in_maps(x, W, b, n_cores=NCORES):
    x = np.asarray(x, dtype=np.float32)
    W = np.asarray(W, dtype=np.float32)
    b = np.asarray(b, dtype=np.float32)
    maps = []
    for c in range(n_cores):
        sl = slice(c * NIS, (c + 1) * NIS)
        maps.append({
            "x": np.ascontiguousarray(x[:, sl, :]),
            "w": np.ascontiguousarray(W[sl]),
            "b": b,
        })
    return maps


def kernel(x, W, b):
    runner = get_runner()
    res = runner(make_in_maps(x, W, b))
    return np.asarray(res[0]["y"], dtype=np.float32)
